# revision 55
# baseline (speedup 1.0000x reference)
"""Trainium2 Bass kernel for the BiDirectionalRNN problem.

Math (matches the fp32 jax reference):
    e = emb[x]                                   # [B, T, 512]
    fwd:  h_t = relu(e_t @ Wf.T + bf + h_{t-1})  # fs[t]
    bwd over reversed e: bs[s]                   # generation order
    xcat = concat_t [fs[t], bs[t]]  -> [B, T*1024]
    h1 = relu(xcat @ W1.T + b1); 4x h = relu(h @ W2.T + b2); out = h @ Wo.T + bo

Strategy (v2: fp16 + hybrid-precision W1):
  * Data-parallel over batch: 1024/8 = 128 samples per NeuronCore.
  * All 16-bit tensors are fp16 (not bf16): same bytes, 4x less rounding
    error. That frees error budget for the hybrid below (full-fp16 model
    rel err 6.7e-4 vs 8.5e-3 for bf16).
  * Host folds embedding + input projection weights into per-direction
    tables WfeB = Wf @ emb.T + bf ([512, 97]). The device builds the
    one-hot of x on the fly (rank-1 matmul replicates the x row over 97
    partitions, DVE is_equal against an arange column), then computes the
    per-step drive terms a = WfeB @ onehot with K=97 matmuls.
  * ScalarE copies each a-GEMM PSUM block into the scan layout
    [p, b*33 + s]; the whole 32-step recurrence h = relu(a + h_prev) runs
    as ONE DVE tensor_tensor_scan per (dir, hid-tile), fp32 state.
  * Hybrid W1 GEMM: scan-state energy grows ~linearly in t, so the first
    TAU=12 time steps carry ~(TAU/T)^2 of the xcat energy. Those k-dims
    go through an fp8 DoubleRow path (1 byte on the wire, 2 k-tiles per
    PE pass): lhsT = X1 = e4m3(32*h) plus a residual term
    X2 = e5m2(32*h - X1) that cancels the device cast error of X1 (same
    product scale -> same PSUM bank); rhs = e4m3(512*W1). The bank is
    drained with scale 2^-14 and folded back into the main PSUM via one
    identity matmul mid-stream. The remaining 20 steps stay fp16.
    Wire: 33.5MB -> 27.3MB. Measured rel err on HW 1.797e-2 (< 2e-2;
    the remaining noise is intrinsic to the fp8 GEMM path and scales
    with the fp8 energy share, which is what bounds TAU).
  * The fp16 W1 part ships in 40 [128,2048] groups ordered (dir, m)-major
    so the GEMM starts right after the first scan; a-scans are software-
    pipelined four steps ahead, x-casts two ahead; fp8 matmuls run
    mid-stream, off the tail (j=7's run during iteration 6).
  * Tail: PE-transpose h1, then 4 x [512,512] + [97,512] in transposed
    (feature-major) layout; biases enter PSUM via rank-1 matmuls; each
    stage uses twin PSUM banks so ScalarE and VectorE drain in parallel.
  * Const/small inputs ride in 4 merged DMAs; the first two fp16 W1
    groups + first fp8 group are issued before them; the tail-only W2/Wo
    weights ship after the W1 stream; the final fp16 group is fetched
    chunk-by-chunk to minimize the end latency.
"""

import numpy as np
import ml_dtypes

F16 = np.float16
E4 = ml_dtypes.float8_e4m3

MOD = 97
HID = 512
T = 32
B = 1024
NCORES = 8
BL = B // NCORES          # 128 batch per core
CL = T + 1                # chain length incl. separator column
FREE = BL * CL            # 4224 scan columns per tile
NEG = -60000.0            # separator; finite in fp16
TAU = 12                  # time steps routed through the fp8 path
NT16 = T - TAU            # fp16 time steps per (dir, m)
G16_PER_J = NT16 // 4     # fp16 W1 groups per j (4 t-chunks each)
W1_GRP = 8 * G16_PER_J    # fp16 W1 DMA groups
NPR = TAU // 2            # fp8 DoubleRow pairs per j
SW = 512.0                # host scale on fp8 W1
SX = 32.0                 # device scale on fp8 scan outputs
SINV = 1.0 / (SW * SX)

_CACHE: dict = {}


def _build(reps=1):
    import concourse.tile as tile
    from concourse import bacc, mybir

    fp32 = mybir.dt.float32
    fp16 = mybir.dt.float16
    fp8 = mybir.dt.float8e4

    nc = bacc.Bacc(
        "TRN2", target_bir_lowering=False, debug=False, num_devices=NCORES
    )

    d = {
        "IDA": nc.dram_tensor("IDA", [128, 129], fp16, kind="ExternalInput").ap(),
        "WFE": nc.dram_tensor("WFE", [MOD, 2 * HID], fp16, kind="ExternalInput").ap(),
        "W1S": nc.dram_tensor("W1S", [W1_GRP, 128, 2048], fp16, kind="ExternalInput").ap(),
        "W8S": nc.dram_tensor("W8S", [8, 128, NPR * 1024], fp8, kind="ExternalInput").ap(),
        "W2O": nc.dram_tensor("W2O", [128, 4 * 512 + 4 * MOD], fp16, kind="ExternalInput").ap(),
        "BIA": nc.dram_tensor("BIA", [1, 1121 + 2 * BL * T], fp16, kind="ExternalInput").ap(),
        "OUT": nc.dram_tensor("OUT", [MOD, BL], fp32, kind="ExternalOutput").ap(),
    }

    with tile.TileContext(nc) as tc:
        for _ in range(reps):
            _emit(tc, d, mybir)

    nc.compile()
    return nc


def _emit(tc, d, mybir):
    nc = tc.nc
    fp32 = mybir.dt.float32
    fp16 = mybir.dt.float16
    fp8 = mybir.dt.float8e4
    fp8e5 = mybir.dt.float8e5
    AF = mybir.ActivationFunctionType
    ALU = mybir.AluOpType
    PM = mybir.MatmulPerfMode

    from contextlib import ExitStack

    with ExitStack() as ctx:
        const = ctx.enter_context(tc.tile_pool(name="const", bufs=1))
        a_pool = ctx.enter_context(tc.tile_pool(name="apool", bufs=2))
        h_pool = ctx.enter_context(tc.tile_pool(name="hpool", bufs=5))
        w1_pool = ctx.enter_context(tc.tile_pool(name="w1pool", bufs=16))
        w8_pool = ctx.enter_context(tc.tile_pool(name="w8pool", bufs=3))
        x1_pool = ctx.enter_context(tc.tile_pool(name="x1pool", bufs=3))
        hp_pool = ctx.enter_context(tc.tile_pool(name="hppool", bufs=3))
        ps_a = ctx.enter_context(tc.tile_pool(name="psa", bufs=2, space="PSUM"))
        ps_h1 = ctx.enter_context(tc.tile_pool(name="psh1", bufs=1, space="PSUM"))
        ps_8 = ctx.enter_context(tc.tile_pool(name="ps8", bufs=1, space="PSUM"))
        ps_l = ctx.enter_context(tc.tile_pool(name="psl", bufs=1, space="PSUM"))
        ps_o = ctx.enter_context(tc.tile_pool(name="pso", bufs=1, space="PSUM"))

        # ---- head: small consts first (the a-phases need WFE asap),
        # then the W1/W8 stream prefetches ----
        wfe = const.tile([MOD, 2 * HID], fp16)
        nc.sync.dma_start(wfe[:], d["WFE"][:])
        w2o = const.tile([128, 4 * 512 + 4 * MOD], fp16)
        w2sb = w2o[:, 0:2048]
        wosb = w2o[:, 2048:2048 + 4 * MOD]
        bia = const.tile([1, 1121 + 2 * BL * T], fp16)
        nc.sync.dma_start(bia[:], d["BIA"])
        b1sb = bia[:, 0:512]
        b2r = bia[:, 512:1024]
        bor = bia[:, 1024:1121]
        xr = bia[:, 1121:1121 + 2 * BL * T]
        ida = const.tile([128, 129], fp16)
        nc.sync.dma_start(ida[:], d["IDA"])
        idsb = ida[:, 0:128]
        arn = ida[:, 128:129]
        w1_pre = {}
        for G in (0, 1):
            w_t = w1_pool.tile([128, 2048], fp16, tag="w_t")
            nc.sync.dma_start(w_t[:], d["W1S"][G])
            w1_pre[G] = w_t
        w8_pre = {}
        for Jp in (0, 1):
            w8_t = w8_pool.tile([128, NPR * 1024], fp8, tag="w8_t")
            nc.sync.dma_start(w8_t[:], d["W8S"][Jp])
            w8_pre[Jp] = w8_t
        ones = const.tile([1, 128], fp16)
        nc.vector.memset(ones[:], 1.0)
        zero = const.tile([128, 1], fp16)
        nc.vector.memset(zero[:], 0.0)
        # one-hot of x, built on device
        ohall = const.tile([MOD, 2 * BL * T], fp16)
        ohsb = [ohall[:, 0:BL * T], ohall[:, BL * T:2 * BL * T]]

        psum_h1 = ps_h1.tile([128, 512], fp32)
        psum_8 = ps_8.tile([128, 512], fp32)
        bias_done = [False]

        asb = {}

        def a_scan_open(j):
            a_sb = a_pool.tile([128, FREE], fp16, tag="a")
            sep = a_sb[:].rearrange("p (b t) -> p b t", t=CL)[:, :, T]
            nc.gpsimd.memset(sep, NEG)
            asb[j] = a_sb

        def a_scan_part(j, q):
            dd, m = j // 4, j % 4
            a_sb = asb[j]
            lhsT = wfe[:, dd * HID + m * 128: dd * HID + m * 128 + 128]
            if m == 0:
                px = ps_a.tile([128, 512], fp32, tag="pa")
                nc.tensor.matmul(
                    px[:MOD, :], ones[:, 0:MOD],
                    xr[:, dd * BL * T + q * 512: dd * BL * T + (q + 1) * 512],
                    start=True, stop=True,
                )
                nc.vector.tensor_tensor(
                    ohsb[dd][:, q * 512:(q + 1) * 512], px[:MOD, :],
                    arn[:MOD, :].broadcast_to([MOD, 512]),
                    op=mybir.AluOpType.is_equal,
                )
            pa = ps_a.tile([128, 512], fp32, tag="pa")
            nc.tensor.matmul(
                pa[:], lhsT, ohsb[dd][:, q * 512:(q + 1) * 512],
                start=True, stop=True,
            )
            av = a_sb[:].rearrange("p (b t) -> p b t", t=CL)[:, 16 * q:16 * q + 16, 0:T]
            nc.scalar.copy(av, pa[:].rearrange("p (b t) -> p b t", t=T))

        def a_scan_close(j):
            h_t = h_pool.tile([128, FREE], fp16, tag="h")
            nc.vector.tensor_tensor_scan(
                h_t[:], asb.pop(j)[:], zero[:].broadcast_to([128, FREE]),
                initial=0.0, op0=ALU.add, op1=ALU.max,
            )
            hs[j] = h_t

        def x_cast(j):
            # fp8 copies of the early-t states for the DoubleRow path:
            # X1[p, t*128 + b] = fp8(32 * h[p, b*33 + t]), t < TAU; the
            # residual X2 = 32*x - X1 captures exactly the device cast
            # error of X1 (same product scale -> same PSUM bank).
            h_t = hs[j]
            hv3 = h_t[:].rearrange("p (b t) -> p t b", t=CL)[:, 0:TAU, :]
            x1 = x1_pool.tile([128, TAU * BL], fp8, tag="x1")
            nc.scalar.activation(
                x1[:].rearrange("p (t b) -> p t b", b=BL), hv3, AF.Copy, scale=SX,
            )
            # residual term X2 = 32*x - X1 captures exactly the device cast
            # error of X1. e5m2 (min normal 2^-14) keeps the small residuals
            # out of the PE's subnormal flush; 2 mantissa bits suffice for a
            # second-order term. Same product scale -> same PSUM bank.
            x2 = x1_pool.tile([128, TAU * BL], fp8e5, tag="x2")
            nc.vector.scalar_tensor_tensor(
                x2[:].rearrange("p (t b) -> p t b", b=BL), hv3, SX,
                x1[:].rearrange("p (t b) -> p t b", b=BL),
                op0=ALU.mult, op1=ALU.subtract,
            )
            xs[j] = (x1, x2)

        hs = {}
        xs = {}

        def a_scan(j):
            a_scan_open(j)
            for q in range(8):
                a_scan_part(j, q)
            a_scan_close(j)

        for j in range(4):
            a_scan(j)
        x_cast(0)
        x_cast(1)

        def fp8_block(j):
            # fp8 DoubleRow matmuls for this j (data prefetched early)
            w8_t = w8_pre.pop(j)
            x1, x2 = xs.pop(j)
            x1v = x1[:].rearrange("p (t b) -> p t b", b=BL)
            x2v = x2[:].rearrange("p (t b) -> p t b", b=BL)
            w8v = w8_t[:].rearrange("p (pr two c) -> p pr two c", two=2, c=512)
            for pr in range(NPR):
                nc.tensor.matmul(
                    psum_8[:], x1v[:, 2 * pr:2 * pr + 2, :], w8v[:, pr],
                    start=(j == 0 and pr == 0), stop=False,
                    perf_mode=PM.DoubleRow,
                )
                nc.tensor.matmul(
                    psum_8[:], x2v[:, 2 * pr:2 * pr + 2, :], w8v[:, pr],
                    start=False, stop=(j == 7 and pr == NPR - 1),
                    perf_mode=PM.DoubleRow,
                )
            if j == 7:
                # fold the fp8 partial back into the main accumulation,
                # just ahead of the final group's stop matmul
                s8 = const.tile([128, 512], fp16)
                nc.scalar.activation(s8[:], psum_8[:], AF.Copy, scale=SINV)
                nc.tensor.matmul(psum_h1[:], idsb, s8[:], start=False, stop=False)

        for j in range(8):
            hv = hs[j][:].rearrange("p (b t) -> p t b", t=CL)
            if j < 7:
                fp8_block(j)
            for G in range(G16_PER_J * j, G16_PER_J * (j + 1)):
                if G == W1_GRP - 1:
                    fp8_block(7)
                w_t = w1_pre.pop(G, None)
                if w_t is None:
                    w_t = w1_pool.tile([128, 2048], fp16, tag="w_t")
                last_grp = G == W1_GRP - 1
                if last_grp:
                    for c in range(4):
                        nc.sync.dma_start(w_t[:, c * 512:(c + 1) * 512],
                                          d["W1S"][G][:, c * 512:(c + 1) * 512])
                elif G > 1:
                    nc.sync.dma_start(w_t[:], d["W1S"][G])
                if not bias_done[0]:
                    nc.tensor.matmul(psum_h1[:], ones[:], b1sb,
                                     start=True, stop=False)
                    bias_done[0] = True
                for c in range(4):
                    t_idx = TAU + (G % G16_PER_J) * 4 + c
                    nc.tensor.matmul(
                        psum_h1[:], hv[:, t_idx, :], w_t[:, c * 512:(c + 1) * 512],
                        start=False, stop=(last_grp and c == 3),
                    )
                if G == G16_PER_J * j:
                    if j + 4 < 8:
                        a_scan(j + 4)
                    if j + 2 < 8:
                        x_cast(j + 2)
                        w8_t2 = w8_pool.tile([128, NPR * 1024], fp8, tag="w8_t")
                        nc.sync.dma_start(w8_t2[:], d["W8S"][j + 2])
                        w8_pre[j + 2] = w8_t2
        # tail-only weights ship after the W1 stream
        nc.sync.dma_start(w2o[:], d["W2O"][:])
        h1sb = const.tile([128, 512], fp16)
        nc.scalar.activation(h1sb[:], psum_h1[:], AF.Relu)

        # ---- transpose h1 to feature-major [512, 128] ----
        pt_a = ps_l.tile([128, 256], fp16, tag="pla")
        pt_b = ps_l.tile([128, 256], fp16, tag="plb")
        cur = hp_pool.tile([128, 512], fp16, tag="hp")
        for m in (0, 1):
            nc.tensor.transpose(
                pt_a[:, (m % 2) * 128:(m % 2) * 128 + 128],
                h1sb[:, m * 128:(m + 1) * 128], idsb[:])
        nc.scalar.copy(cur[:, 0:256], pt_a[:])
        for m in (2, 3):
            nc.tensor.transpose(
                pt_b[:, (m % 2) * 128:(m % 2) * 128 + 128],
                h1sb[:, m * 128:(m + 1) * 128], idsb[:])
        nc.vector.tensor_copy(cur[:, 256:512], pt_b[:])

        # ---- 4 x (h = relu(W2 @ h' + b2)), feature-major, col block = m ----
        for _L in range(4):
            pl_a = ps_l.tile([128, 256], fp32, tag="pla")
            pl_b = ps_l.tile([128, 256], fp32, tag="plb")
            for m in range(4):
                pl = pl_a if m < 2 else pl_b
                col = (m % 2) * 128
                nc.tensor.matmul(
                    pl[:, col:col + 128],
                    b2r[:, m * 128:(m + 1) * 128], ones[:],
                    start=True, stop=False,
                )
                for k in range(4):
                    nc.tensor.matmul(
                        pl[:, col:col + 128],
                        w2sb[:, k * 512 + m * 128: k * 512 + m * 128 + 128],
                        cur[:, k * 128:(k + 1) * 128],
                        start=False, stop=(k == 3),
                    )
            hq = hp_pool.tile([128, 512], fp16, tag="hp")
            nc.scalar.activation(hq[:, 0:256], pl_a[:], AF.Relu)
            nc.vector.tensor_scalar_max(hq[:, 256:512], pl_b[:], 0.0)
            cur = hq

        # ---- output head: out' = Wo @ h' + bo  -> [97, 128] ----
        po = ps_o.tile([MOD, 128], fp32, tag="po")
        nc.tensor.matmul(po[:], bor, ones[:], start=True, stop=False)
        for k in range(4):
            nc.tensor.matmul(
                po[:], wosb[:, k * MOD:(k + 1) * MOD], cur[:, k * 128:(k + 1) * 128],
                start=False, stop=(k == 3),
            )
        osb = const.tile([MOD, BL], fp32)
        nc.scalar.copy(osb[:], po[:])
        nc.sync.dma_start(d["OUT"], osb[:])


def _host_prep(inputs):
    x = np.asarray(inputs["x"]).astype(np.int64)          # [B, T]
    emb = np.asarray(inputs["emb"], np.float32)           # [97, 512]
    Wf = np.asarray(inputs["Wf"], np.float32)
    bf = np.asarray(inputs["bf"], np.float32)
    Wb = np.asarray(inputs["Wb"], np.float32)
    bb = np.asarray(inputs["bb"], np.float32)
    W1 = np.asarray(inputs["W1"], np.float32)             # [512, 32768]
    b1 = np.asarray(inputs["b1"], np.float32)
    W2 = np.asarray(inputs["W2"], np.float32)
    b2 = np.asarray(inputs["b2"], np.float32)
    Wo = np.asarray(inputs["Wo"], np.float32)             # [97, 512]
    bo = np.asarray(inputs["bo"], np.float32)

    WFE = np.ascontiguousarray(np.stack([
        (Wf @ emb.T + bf[:, None]).T,                     # [97, 512]
        (Wb @ emb.T + bb[:, None]).T,
    ]).transpose(1, 0, 2).reshape(MOD, 2 * HID)).astype(F16)

    xc = x.reshape(NCORES, BL, T)
    XR = np.concatenate([
        xc.reshape(NCORES, BL * T), xc[:, :, ::-1].reshape(NCORES, BL * T)
    ], axis=1).astype(F16)                                # [NC, 8192]
    IDA = np.concatenate([
        np.eye(128, dtype=np.float32),
        np.arange(128, dtype=np.float32).reshape(128, 1),
    ], axis=1).astype(F16)

    # W1.T row layout is [t, d, m, p]-major (xcat col = t*1024 + d*512 + m*128)
    W1t = W1.T.reshape(T, 2, 4, 128, 512)                 # [t, d, m, p, col]
    # fp16 part: t in [TAU, 32): group G = (d, m, tg) holds t-chunks
    # t = TAU + 4*tg .. TAU + 4*tg + 3 side by side
    W1S = np.ascontiguousarray(
        W1t[TAU:]                                         # [NT16, d, m, p, col]
        .reshape(G16_PER_J, 4, 2, 4, 128, 512)            # [tg, tc, d, m, p, col]
        .transpose(2, 3, 0, 4, 1, 5)                      # [d, m, tg, p, tc, col]
        .reshape(W1_GRP, 128, 2048)
    ).astype(F16)
    # fp8 part: t in [0, TAU) as DoubleRow pairs, scaled by SW
    W8S = np.ascontiguousarray(
        (W1t[:TAU] * SW)                                  # [TAU, d, m, p, col]
        .reshape(NPR, 2, 2, 4, 128, 512)                  # [pr, two, d, m, p, col]
        .transpose(2, 3, 4, 0, 1, 5)                      # [d, m, p, pr, two, col]
        .reshape(8, 128, NPR * 1024)
    ).astype(E4)
    W2S = np.ascontiguousarray(W2.T.reshape(4, 128, 512).transpose(1, 0, 2).reshape(128, 2048)).astype(F16)
    WOS = np.ascontiguousarray(Wo.T.reshape(4, 128, MOD).transpose(1, 0, 2).reshape(128, 4 * MOD)).astype(F16)
    W2O = np.concatenate([W2S, WOS], axis=1)
    BIAH = np.concatenate([b1, b2, bo]).astype(F16)       # [1121]

    shared = {"WFE": WFE, "W1S": W1S, "W8S": W8S, "W2O": W2O, "IDA": IDA}
    in_maps = [
        dict(shared, BIA=np.concatenate([BIAH, XR[c]]).reshape(1, -1))
        for c in range(NCORES)
    ]
    return in_maps


def _get_nc():
    if "nc" not in _CACHE:
        _CACHE["nc"] = _build()
    return _CACHE["nc"]


def kernel(**inputs):
    from concourse.bass_utils import run_bass_kernel_spmd

    nc = _get_nc()
    in_maps = _host_prep(inputs)
    res = run_bass_kernel_spmd(nc, in_maps, list(range(NCORES)))
    outs = [np.asarray(res.results[c]["OUT"], np.float32) for c in range(NCORES)]
    return np.ascontiguousarray(np.concatenate([o.T for o in outs], axis=0))  # [1024, 97]


# revision 58
# speedup vs baseline: 1.0016x; 1.0016x over previous
"""Trainium2 Bass kernel for the BiDirectionalRNN problem.

Math (matches the fp32 jax reference):
    e = emb[x]                                   # [B, T, 512]
    fwd:  h_t = relu(e_t @ Wf.T + bf + h_{t-1})  # fs[t]
    bwd over reversed e: bs[s]                   # generation order
    xcat = concat_t [fs[t], bs[t]]  -> [B, T*1024]
    h1 = relu(xcat @ W1.T + b1); 4x h = relu(h @ W2.T + b2); out = h @ Wo.T + bo

Strategy (v2: fp16 + hybrid-precision W1):
  * Data-parallel over batch: 1024/8 = 128 samples per NeuronCore.
  * All 16-bit tensors are fp16 (not bf16): same bytes, 4x less rounding
    error. That frees error budget for the hybrid below (full-fp16 model
    rel err 6.7e-4 vs 8.5e-3 for bf16).
  * Host folds embedding + input projection weights into per-direction
    tables WfeB = Wf @ emb.T + bf ([512, 97]). The device builds the
    one-hot of x on the fly (rank-1 matmul replicates the x row over 97
    partitions, DVE is_equal against an arange column), then computes the
    per-step drive terms a = WfeB @ onehot with K=97 matmuls.
  * ScalarE copies each a-GEMM PSUM block into the scan layout
    [p, b*33 + s]; the whole 32-step recurrence h = relu(a + h_prev) runs
    as ONE DVE tensor_tensor_scan per (dir, hid-tile), fp32 state.
  * Hybrid W1 GEMM: scan-state energy grows ~linearly in t, so the first
    TAU=12 time steps carry ~(TAU/T)^2 of the xcat energy. Those k-dims
    go through an fp8 DoubleRow path (1 byte on the wire, 2 k-tiles per
    PE pass): lhsT = X1 = e4m3(32*h) plus a residual term
    X2 = e5m2(32*h - X1) that cancels the device cast error of X1 (same
    product scale -> same PSUM bank); rhs = e4m3(512*W1). The bank is
    drained with scale 2^-14 and folded back into the main PSUM via one
    identity matmul mid-stream. The remaining 20 steps stay fp16.
    Wire: 33.5MB -> 27.3MB. Measured rel err on HW 1.797e-2 (< 2e-2;
    the remaining noise is intrinsic to the fp8 GEMM path and scales
    with the fp8 energy share, which is what bounds TAU).
  * The fp16 W1 part ships in 40 [128,2048] groups ordered (dir, m)-major
    so the GEMM starts right after the first scan; a-scans are software-
    pipelined four steps ahead, x-casts two ahead; fp8 matmuls run
    mid-stream, off the tail (j=7's run during iteration 6).
  * Tail: PE-transpose h1, then 4 x [512,512] + [97,512] in transposed
    (feature-major) layout; biases enter PSUM via rank-1 matmuls; each
    stage uses twin PSUM banks so ScalarE and VectorE drain in parallel.
  * Const/small inputs ride in 4 merged DMAs; the first two fp16 W1
    groups + first fp8 group are issued before them; the tail-only W2/Wo
    weights ship after the W1 stream; the final fp16 group is fetched
    chunk-by-chunk to minimize the end latency.
"""

import numpy as np
import ml_dtypes

F16 = np.float16
E4 = ml_dtypes.float8_e4m3

MOD = 97
HID = 512
T = 32
B = 1024
NCORES = 8
BL = B // NCORES          # 128 batch per core
CL = T + 1                # chain length incl. separator column
FREE = BL * CL            # 4224 scan columns per tile
NEG = -60000.0            # separator; finite in fp16
TAU = 12                  # time steps routed through the fp8 path
NT16 = T - TAU            # fp16 time steps per (dir, m)
G16_PER_J = NT16 // 4     # fp16 W1 groups per j (4 t-chunks each)
W1_GRP = 8 * G16_PER_J    # fp16 W1 DMA groups
NPR = TAU // 2            # fp8 DoubleRow pairs per j
SW = 512.0                # host scale on fp8 W1
SX = 32.0                 # device scale on fp8 scan outputs
SINV = 1.0 / (SW * SX)

_CACHE: dict = {}


def _build(reps=1):
    import concourse.tile as tile
    from concourse import bacc, mybir

    fp32 = mybir.dt.float32
    fp16 = mybir.dt.float16
    fp8 = mybir.dt.float8e4

    nc = bacc.Bacc(
        "TRN2", target_bir_lowering=False, debug=False, num_devices=NCORES
    )

    d = {
        "WFI": nc.dram_tensor("WFI", [128, 2 * HID + 129], fp16, kind="ExternalInput").ap(),
        "W1S": nc.dram_tensor("W1S", [W1_GRP, 128, 2048], fp16, kind="ExternalInput").ap(),
        "W8S": nc.dram_tensor("W8S", [8, 128, NPR * 1024], fp8, kind="ExternalInput").ap(),
        "W2O": nc.dram_tensor("W2O", [128, 4 * 512 + 4 * MOD], fp16, kind="ExternalInput").ap(),
        "BIA": nc.dram_tensor("BIA", [1, 1121 + 2 * BL * T], fp16, kind="ExternalInput").ap(),
        "OUT": nc.dram_tensor("OUT", [MOD, BL], fp32, kind="ExternalOutput").ap(),
    }

    with tile.TileContext(nc) as tc:
        for _ in range(reps):
            _emit(tc, d, mybir)

    nc.compile()
    return nc


def _emit(tc, d, mybir):
    nc = tc.nc
    fp32 = mybir.dt.float32
    fp16 = mybir.dt.float16
    fp8 = mybir.dt.float8e4
    fp8e5 = mybir.dt.float8e5
    AF = mybir.ActivationFunctionType
    ALU = mybir.AluOpType
    PM = mybir.MatmulPerfMode

    from contextlib import ExitStack

    with ExitStack() as ctx:
        const = ctx.enter_context(tc.tile_pool(name="const", bufs=1))
        a_pool = ctx.enter_context(tc.tile_pool(name="apool", bufs=2))
        h_pool = ctx.enter_context(tc.tile_pool(name="hpool", bufs=5))
        w1_pool = ctx.enter_context(tc.tile_pool(name="w1pool", bufs=16))
        w8_pool = ctx.enter_context(tc.tile_pool(name="w8pool", bufs=3))
        x1_pool = ctx.enter_context(tc.tile_pool(name="x1pool", bufs=3))
        hp_pool = ctx.enter_context(tc.tile_pool(name="hppool", bufs=3))
        ps_a = ctx.enter_context(tc.tile_pool(name="psa", bufs=2, space="PSUM"))
        ps_h1 = ctx.enter_context(tc.tile_pool(name="psh1", bufs=1, space="PSUM"))
        ps_8 = ctx.enter_context(tc.tile_pool(name="ps8", bufs=1, space="PSUM"))
        ps_l = ctx.enter_context(tc.tile_pool(name="psl", bufs=1, space="PSUM"))
        ps_o = ctx.enter_context(tc.tile_pool(name="pso", bufs=1, space="PSUM"))

        # ---- head: small consts first (the a-phases need WFE asap),
        # then the W1/W8 stream prefetches ----
        wfi = const.tile([128, 2 * HID + 129], fp16)
        nc.sync.dma_start(wfi[:], d["WFI"][:])
        wfe = wfi[:MOD, 0:2 * HID]
        idsb = wfi[:, 2 * HID:2 * HID + 128]
        arn = wfi[:, 2 * HID + 128:2 * HID + 129]
        w2o = const.tile([128, 4 * 512 + 4 * MOD], fp16)
        w2sb = w2o[:, 0:2048]
        wosb = w2o[:, 2048:2048 + 4 * MOD]
        bia = const.tile([1, 1121 + 2 * BL * T], fp16)
        nc.sync.dma_start(bia[:], d["BIA"])
        b1sb = bia[:, 0:512]
        b2r = bia[:, 512:1024]
        bor = bia[:, 1024:1121]
        xr = bia[:, 1121:1121 + 2 * BL * T]
        w1_pre = {}
        for G in (0, 1):
            w_t = w1_pool.tile([128, 2048], fp16, tag="w_t")
            nc.sync.dma_start(w_t[:], d["W1S"][G])
            w1_pre[G] = w_t
        w8_pre = {}
        for Jp in (0, 1):
            w8_t = w8_pool.tile([128, NPR * 1024], fp8, tag="w8_t")
            nc.sync.dma_start(w8_t[:], d["W8S"][Jp])
            w8_pre[Jp] = w8_t
        ones = const.tile([1, 128], fp16)
        nc.vector.memset(ones[:], 1.0)
        zero = const.tile([128, 1], fp16)
        nc.vector.memset(zero[:], 0.0)
        # one-hot of x, built on device
        ohall = const.tile([MOD, 2 * BL * T], fp16)
        ohsb = [ohall[:, 0:BL * T], ohall[:, BL * T:2 * BL * T]]

        psum_h1 = ps_h1.tile([128, 512], fp32)
        psum_8 = ps_8.tile([128, 512], fp32)
        bias_done = [False]

        asb = {}

        def a_scan_open(j):
            a_sb = a_pool.tile([128, FREE], fp16, tag="a")
            sep = a_sb[:].rearrange("p (b t) -> p b t", t=CL)[:, :, T]
            nc.gpsimd.memset(sep, NEG)
            asb[j] = a_sb

        def a_scan_part(j, q):
            dd, m = j // 4, j % 4
            a_sb = asb[j]
            lhsT = wfe[:, dd * HID + m * 128: dd * HID + m * 128 + 128]
            if m == 0:
                px = ps_a.tile([128, 512], fp32, tag="pa")
                nc.tensor.matmul(
                    px[:MOD, :], ones[:, 0:MOD],
                    xr[:, dd * BL * T + q * 512: dd * BL * T + (q + 1) * 512],
                    start=True, stop=True,
                )
                nc.vector.tensor_tensor(
                    ohsb[dd][:, q * 512:(q + 1) * 512], px[:MOD, :],
                    arn[:MOD, :].broadcast_to([MOD, 512]),
                    op=mybir.AluOpType.is_equal,
                )
            pa = ps_a.tile([128, 512], fp32, tag="pa")
            nc.tensor.matmul(
                pa[:], lhsT, ohsb[dd][:, q * 512:(q + 1) * 512],
                start=True, stop=True,
            )
            av = a_sb[:].rearrange("p (b t) -> p b t", t=CL)[:, 16 * q:16 * q + 16, 0:T]
            nc.scalar.copy(av, pa[:].rearrange("p (b t) -> p b t", t=T))

        def a_scan_close(j):
            h_t = h_pool.tile([128, FREE], fp16, tag="h")
            nc.vector.tensor_tensor_scan(
                h_t[:], asb.pop(j)[:], zero[:].broadcast_to([128, FREE]),
                initial=0.0, op0=ALU.add, op1=ALU.max,
            )
            hs[j] = h_t

        def x_cast(j):
            # fp8 copies of the early-t states for the DoubleRow path:
            # X1[p, t*128 + b] = fp8(32 * h[p, b*33 + t]), t < TAU; the
            # residual X2 = 32*x - X1 captures exactly the device cast
            # error of X1 (same product scale -> same PSUM bank).
            h_t = hs[j]
            hv3 = h_t[:].rearrange("p (b t) -> p t b", t=CL)[:, 0:TAU, :]
            x1 = x1_pool.tile([128, TAU * BL], fp8, tag="x1")
            nc.scalar.activation(
                x1[:].rearrange("p (t b) -> p t b", b=BL), hv3, AF.Copy, scale=SX,
            )
            # residual term X2 = 32*x - X1 captures exactly the device cast
            # error of X1. e5m2 (min normal 2^-14) keeps the small residuals
            # out of the PE's subnormal flush; 2 mantissa bits suffice for a
            # second-order term. Same product scale -> same PSUM bank.
            x2 = x1_pool.tile([128, TAU * BL], fp8e5, tag="x2")
            nc.vector.scalar_tensor_tensor(
                x2[:].rearrange("p (t b) -> p t b", b=BL), hv3, SX,
                x1[:].rearrange("p (t b) -> p t b", b=BL),
                op0=ALU.mult, op1=ALU.subtract,
            )
            xs[j] = (x1, x2)

        hs = {}
        xs = {}

        def a_scan(j):
            a_scan_open(j)
            for q in range(8):
                a_scan_part(j, q)
            a_scan_close(j)

        for j in range(4):
            a_scan(j)
        x_cast(0)
        x_cast(1)

        def fp8_block(j):
            # fp8 DoubleRow matmuls for this j (data prefetched early)
            w8_t = w8_pre.pop(j)
            x1, x2 = xs.pop(j)
            x1v = x1[:].rearrange("p (t b) -> p t b", b=BL)
            x2v = x2[:].rearrange("p (t b) -> p t b", b=BL)
            w8v = w8_t[:].rearrange("p (pr two c) -> p pr two c", two=2, c=512)
            for pr in range(NPR):
                nc.tensor.matmul(
                    psum_8[:], x1v[:, 2 * pr:2 * pr + 2, :], w8v[:, pr],
                    start=(j == 0 and pr == 0), stop=False,
                    perf_mode=PM.DoubleRow,
                )
                nc.tensor.matmul(
                    psum_8[:], x2v[:, 2 * pr:2 * pr + 2, :], w8v[:, pr],
                    start=False, stop=(j == 7 and pr == NPR - 1),
                    perf_mode=PM.DoubleRow,
                )
            if j == 7:
                # fold the fp8 partial back into the main accumulation,
                # just ahead of the final group's stop matmul
                s8 = const.tile([128, 512], fp16)
                nc.scalar.activation(s8[:], psum_8[:], AF.Copy, scale=SINV)
                nc.tensor.matmul(psum_h1[:], idsb, s8[:], start=False, stop=False)

        for j in range(8):
            hv = hs[j][:].rearrange("p (b t) -> p t b", t=CL)
            if j < 7:
                fp8_block(j)
            for G in range(G16_PER_J * j, G16_PER_J * (j + 1)):
                if G == W1_GRP - 1:
                    fp8_block(7)
                w_t = w1_pre.pop(G, None)
                if w_t is None:
                    w_t = w1_pool.tile([128, 2048], fp16, tag="w_t")
                last_grp = G == W1_GRP - 1
                if last_grp:
                    for c in range(4):
                        nc.sync.dma_start(w_t[:, c * 512:(c + 1) * 512],
                                          d["W1S"][G][:, c * 512:(c + 1) * 512])
                elif G > 1:
                    nc.sync.dma_start(w_t[:], d["W1S"][G])
                if not bias_done[0]:
                    nc.tensor.matmul(psum_h1[:], ones[:], b1sb,
                                     start=True, stop=False)
                    bias_done[0] = True
                for c in range(4):
                    t_idx = TAU + (G % G16_PER_J) * 4 + c
                    nc.tensor.matmul(
                        psum_h1[:], hv[:, t_idx, :], w_t[:, c * 512:(c + 1) * 512],
                        start=False, stop=(last_grp and c == 3),
                    )
                if G == G16_PER_J * j:
                    if j + 4 < 8:
                        a_scan(j + 4)
                    if j + 2 < 8:
                        x_cast(j + 2)
                        w8_t2 = w8_pool.tile([128, NPR * 1024], fp8, tag="w8_t")
                        nc.sync.dma_start(w8_t2[:], d["W8S"][j + 2])
                        w8_pre[j + 2] = w8_t2
        # tail-only weights ship after the W1 stream
        nc.sync.dma_start(w2o[:], d["W2O"][:])
        h1sb = const.tile([128, 512], fp16)
        nc.scalar.activation(h1sb[:], psum_h1[:], AF.Relu)

        # ---- transpose h1 to feature-major [512, 128] ----
        pt_a = ps_l.tile([128, 256], fp16, tag="pla")
        pt_b = ps_l.tile([128, 256], fp16, tag="plb")
        cur = hp_pool.tile([128, 512], fp16, tag="hp")
        for m in (0, 1):
            nc.tensor.transpose(
                pt_a[:, (m % 2) * 128:(m % 2) * 128 + 128],
                h1sb[:, m * 128:(m + 1) * 128], idsb[:])
        nc.scalar.copy(cur[:, 0:256], pt_a[:])
        for m in (2, 3):
            nc.tensor.transpose(
                pt_b[:, (m % 2) * 128:(m % 2) * 128 + 128],
                h1sb[:, m * 128:(m + 1) * 128], idsb[:])
        nc.vector.tensor_copy(cur[:, 256:512], pt_b[:])

        # ---- 4 x (h = relu(W2 @ h' + b2)), feature-major, col block = m ----
        for _L in range(4):
            pl_a = ps_l.tile([128, 256], fp32, tag="pla")
            pl_b = ps_l.tile([128, 256], fp32, tag="plb")
            for m in range(4):
                pl = pl_a if m < 2 else pl_b
                col = (m % 2) * 128
                nc.tensor.matmul(
                    pl[:, col:col + 128],
                    b2r[:, m * 128:(m + 1) * 128], ones[:],
                    start=True, stop=False,
                )
                for k in range(4):
                    nc.tensor.matmul(
                        pl[:, col:col + 128],
                        w2sb[:, k * 512 + m * 128: k * 512 + m * 128 + 128],
                        cur[:, k * 128:(k + 1) * 128],
                        start=False, stop=(k == 3),
                    )
            hq = hp_pool.tile([128, 512], fp16, tag="hp")
            nc.scalar.activation(hq[:, 0:256], pl_a[:], AF.Relu)
            nc.vector.tensor_scalar_max(hq[:, 256:512], pl_b[:], 0.0)
            cur = hq

        # ---- output head: out' = Wo @ h' + bo  -> [97, 128] ----
        po = ps_o.tile([MOD, 128], fp32, tag="po")
        nc.tensor.matmul(po[:], bor, ones[:], start=True, stop=False)
        for k in range(4):
            nc.tensor.matmul(
                po[:], wosb[:, k * MOD:(k + 1) * MOD], cur[:, k * 128:(k + 1) * 128],
                start=False, stop=(k == 3),
            )
        osb = const.tile([MOD, BL], fp32)
        nc.scalar.copy(osb[:], po[:])
        nc.sync.dma_start(d["OUT"], osb[:])


def _host_prep(inputs):
    x = np.asarray(inputs["x"]).astype(np.int64)          # [B, T]
    emb = np.asarray(inputs["emb"], np.float32)           # [97, 512]
    Wf = np.asarray(inputs["Wf"], np.float32)
    bf = np.asarray(inputs["bf"], np.float32)
    Wb = np.asarray(inputs["Wb"], np.float32)
    bb = np.asarray(inputs["bb"], np.float32)
    W1 = np.asarray(inputs["W1"], np.float32)             # [512, 32768]
    b1 = np.asarray(inputs["b1"], np.float32)
    W2 = np.asarray(inputs["W2"], np.float32)
    b2 = np.asarray(inputs["b2"], np.float32)
    Wo = np.asarray(inputs["Wo"], np.float32)             # [97, 512]
    bo = np.asarray(inputs["bo"], np.float32)

    WFE = np.ascontiguousarray(np.stack([
        (Wf @ emb.T + bf[:, None]).T,                     # [97, 512]
        (Wb @ emb.T + bb[:, None]).T,
    ]).transpose(1, 0, 2).reshape(MOD, 2 * HID)).astype(F16)

    xc = x.reshape(NCORES, BL, T)
    XR = np.concatenate([
        xc.reshape(NCORES, BL * T), xc[:, :, ::-1].reshape(NCORES, BL * T)
    ], axis=1).astype(F16)                                # [NC, 8192]
    IDA = np.concatenate([
        np.eye(128, dtype=np.float32),
        np.arange(128, dtype=np.float32).reshape(128, 1),
    ], axis=1).astype(F16)

    # W1.T row layout is [t, d, m, p]-major (xcat col = t*1024 + d*512 + m*128)
    W1t = W1.T.reshape(T, 2, 4, 128, 512)                 # [t, d, m, p, col]
    # fp16 part: t in [TAU, 32): group G = (d, m, tg) holds t-chunks
    # t = TAU + 4*tg .. TAU + 4*tg + 3 side by side
    W1S = np.ascontiguousarray(
        W1t[TAU:]                                         # [NT16, d, m, p, col]
        .reshape(G16_PER_J, 4, 2, 4, 128, 512)            # [tg, tc, d, m, p, col]
        .transpose(2, 3, 0, 4, 1, 5)                      # [d, m, tg, p, tc, col]
        .reshape(W1_GRP, 128, 2048)
    ).astype(F16)
    # fp8 part: t in [0, TAU) as DoubleRow pairs, scaled by SW
    W8S = np.ascontiguousarray(
        (W1t[:TAU] * SW)                                  # [TAU, d, m, p, col]
        .reshape(NPR, 2, 2, 4, 128, 512)                  # [pr, two, d, m, p, col]
        .transpose(2, 3, 4, 0, 1, 5)                      # [d, m, p, pr, two, col]
        .reshape(8, 128, NPR * 1024)
    ).astype(E4)
    W2S = np.ascontiguousarray(W2.T.reshape(4, 128, 512).transpose(1, 0, 2).reshape(128, 2048)).astype(F16)
    WOS = np.ascontiguousarray(Wo.T.reshape(4, 128, MOD).transpose(1, 0, 2).reshape(128, 4 * MOD)).astype(F16)
    W2O = np.concatenate([W2S, WOS], axis=1)
    BIAH = np.concatenate([b1, b2, bo]).astype(F16)       # [1121]

    WFI = np.concatenate([
        np.concatenate([WFE, np.zeros((128 - MOD, 2 * HID), F16)], axis=0), IDA,
    ], axis=1)
    shared = {"WFI": WFI, "W1S": W1S, "W8S": W8S, "W2O": W2O}
    in_maps = [
        dict(shared, BIA=np.concatenate([BIAH, XR[c]]).reshape(1, -1))
        for c in range(NCORES)
    ]
    return in_maps


def _get_nc():
    if "nc" not in _CACHE:
        _CACHE["nc"] = _build()
    return _CACHE["nc"]


def kernel(**inputs):
    from concourse.bass_utils import run_bass_kernel_spmd

    nc = _get_nc()
    in_maps = _host_prep(inputs)
    res = run_bass_kernel_spmd(nc, in_maps, list(range(NCORES)))
    outs = [np.asarray(res.results[c]["OUT"], np.float32) for c in range(NCORES)]
    return np.ascontiguousarray(np.concatenate([o.T for o in outs], axis=0))  # [1024, 97]


# revision 63
# speedup vs baseline: 1.0058x; 1.0042x over previous
"""Trainium2 Bass kernel for the BiDirectionalRNN problem.

Math (matches the fp32 jax reference):
    e = emb[x]                                   # [B, T, 512]
    fwd:  h_t = relu(e_t @ Wf.T + bf + h_{t-1})  # fs[t]
    bwd over reversed e: bs[s]                   # generation order
    xcat = concat_t [fs[t], bs[t]]  -> [B, T*1024]
    h1 = relu(xcat @ W1.T + b1); 4x h = relu(h @ W2.T + b2); out = h @ Wo.T + bo

Strategy (v2: fp16 + hybrid-precision W1):
  * Data-parallel over batch: 1024/8 = 128 samples per NeuronCore.
  * All 16-bit tensors are fp16 (not bf16): same bytes, 4x less rounding
    error. That frees error budget for the hybrid below (full-fp16 model
    rel err 6.7e-4 vs 8.5e-3 for bf16).
  * Host folds embedding + input projection weights into per-direction
    tables WfeB = Wf @ emb.T + bf ([512, 97]). The device builds the
    one-hot of x on the fly (rank-1 matmul replicates the x row over 97
    partitions, DVE is_equal against an arange column), then computes the
    per-step drive terms a = WfeB @ onehot with K=97 matmuls.
  * ScalarE copies each a-GEMM PSUM block into the scan layout
    [p, b*33 + s]; the whole 32-step recurrence h = relu(a + h_prev) runs
    as ONE DVE tensor_tensor_scan per (dir, hid-tile), fp32 state.
  * Hybrid W1 GEMM: scan-state energy grows ~linearly in t, so the first
    TAU=12 time steps carry ~(TAU/T)^2 of the xcat energy. Those k-dims
    go through an fp8 DoubleRow path (1 byte on the wire, 2 k-tiles per
    PE pass): lhsT = X1 = e4m3(32*h) plus a residual term
    X2 = e5m2(32*h - X1) that cancels the device cast error of X1 (same
    product scale -> same PSUM bank); rhs = e4m3(512*W1). The bank is
    drained with scale 2^-14 and folded back into the main PSUM via one
    identity matmul mid-stream. The remaining 20 steps stay fp16.
    Wire: 33.5MB -> 27.3MB. Measured rel err on HW 1.797e-2 (< 2e-2;
    the remaining noise is intrinsic to the fp8 GEMM path and scales
    with the fp8 energy share, which is what bounds TAU).
  * The fp16 W1 part ships in 40 [128,2048] groups ordered (dir, m)-major
    so the GEMM starts right after the first scan; a-scans are software-
    pipelined four steps ahead, x-casts two ahead; fp8 matmuls run
    mid-stream, off the tail (j=7's run during iteration 6).
  * Tail: PE-transpose h1, then 4 x [512,512] + [97,512] in transposed
    (feature-major) layout; biases enter PSUM via rank-1 matmuls; each
    stage uses twin PSUM banks so ScalarE and VectorE drain in parallel.
  * Const/small inputs ride in 4 merged DMAs; the first two fp16 W1
    groups + first fp8 group are issued before them; the tail-only W2/Wo
    weights ship after the W1 stream; the final fp16 group is fetched
    chunk-by-chunk to minimize the end latency.
"""

import numpy as np
import ml_dtypes

F16 = np.float16
E4 = ml_dtypes.float8_e4m3

MOD = 97
HID = 512
T = 32
B = 1024
NCORES = 8
BL = B // NCORES          # 128 batch per core
CL = T + 1                # chain length incl. separator column
FREE = BL * CL            # 4224 scan columns per tile
NEG = -60000.0            # separator; finite in fp16
TAU = 12                  # time steps routed through the fp8 path
NT16 = T - TAU            # fp16 time steps per (dir, m)
G16_PER_J = NT16 // 4     # fp16 W1 groups per j (4 t-chunks each)
W1_GRP = 8 * G16_PER_J    # fp16 W1 DMA groups
NPR = TAU // 2            # fp8 DoubleRow pairs per j
SW = 512.0                # host scale on fp8 W1
SX = 32.0                 # device scale on fp8 scan outputs
SINV = 1.0 / (SW * SX)

_CACHE: dict = {}


def _build(reps=1):
    import concourse.tile as tile
    from concourse import bacc, mybir

    fp32 = mybir.dt.float32
    fp16 = mybir.dt.float16
    fp8 = mybir.dt.float8e4

    nc = bacc.Bacc(
        "TRN2", target_bir_lowering=False, debug=False, num_devices=NCORES
    )

    d = {
        "WFI": nc.dram_tensor("WFI", [128, 2 * HID + 129], fp16, kind="ExternalInput").ap(),
        "W1S": nc.dram_tensor("W1S", [W1_GRP, 128, 2048], fp16, kind="ExternalInput").ap(),
        "W8S": nc.dram_tensor("W8S", [8, 128, NPR * 1024], fp8, kind="ExternalInput").ap(),
        "W2O": nc.dram_tensor("W2O", [128, 4 * 512 + 4 * MOD], fp16, kind="ExternalInput").ap(),
        "BIA": nc.dram_tensor("BIA", [1, 1121 + 2 * BL * T], fp16, kind="ExternalInput").ap(),
        "OUT": nc.dram_tensor("OUT", [MOD, BL], fp32, kind="ExternalOutput").ap(),
    }

    with tile.TileContext(nc) as tc:
        for _ in range(reps):
            _emit(tc, d, mybir)

    nc.compile()
    return nc


def _emit(tc, d, mybir):
    nc = tc.nc
    fp32 = mybir.dt.float32
    fp16 = mybir.dt.float16
    fp8 = mybir.dt.float8e4
    fp8e5 = mybir.dt.float8e5
    AF = mybir.ActivationFunctionType
    ALU = mybir.AluOpType
    PM = mybir.MatmulPerfMode

    from contextlib import ExitStack

    with ExitStack() as ctx:
        const = ctx.enter_context(tc.tile_pool(name="const", bufs=1))
        a_pool = ctx.enter_context(tc.tile_pool(name="apool", bufs=2))
        h_pool = ctx.enter_context(tc.tile_pool(name="hpool", bufs=5))
        w1_pool = ctx.enter_context(tc.tile_pool(name="w1pool", bufs=16))
        w8_pool = ctx.enter_context(tc.tile_pool(name="w8pool", bufs=4))
        x1_pool = ctx.enter_context(tc.tile_pool(name="x1pool", bufs=3))
        hp_pool = ctx.enter_context(tc.tile_pool(name="hppool", bufs=3))
        ps_a = ctx.enter_context(tc.tile_pool(name="psa", bufs=2, space="PSUM"))
        ps_h1 = ctx.enter_context(tc.tile_pool(name="psh1", bufs=1, space="PSUM"))
        ps_8 = ctx.enter_context(tc.tile_pool(name="ps8", bufs=1, space="PSUM"))
        ps_l = ctx.enter_context(tc.tile_pool(name="psl", bufs=1, space="PSUM"))
        ps_o = ctx.enter_context(tc.tile_pool(name="pso", bufs=1, space="PSUM"))

        # ---- head: small consts first (the a-phases need WFE asap),
        # then the W1/W8 stream prefetches ----
        wfi = const.tile([128, 2 * HID + 129], fp16)
        nc.sync.dma_start(wfi[:], d["WFI"][:])
        wfe = wfi[:MOD, 0:2 * HID]
        idsb = wfi[:, 2 * HID:2 * HID + 128]
        arn = wfi[:, 2 * HID + 128:2 * HID + 129]
        w2o = const.tile([128, 4 * 512 + 4 * MOD], fp16)
        w2sb = w2o[:, 0:2048]
        wosb = w2o[:, 2048:2048 + 4 * MOD]
        bia = const.tile([1, 1121 + 2 * BL * T], fp16)
        nc.sync.dma_start(bia[:], d["BIA"])
        b1sb = bia[:, 0:512]
        b2r = bia[:, 512:1024]
        bor = bia[:, 1024:1121]
        xr = bia[:, 1121:1121 + 2 * BL * T]
        w1_pre = {}
        for G in (0, 1):
            w_t = w1_pool.tile([128, 2048], fp16, tag="w_t")
            nc.sync.dma_start(w_t[:], d["W1S"][G])
            w1_pre[G] = w_t
        w8_pre = {}
        for Jp in (0, 1):
            w8_t = w8_pool.tile([128, NPR * 1024], fp8, tag="w8_t")
            nc.sync.dma_start(w8_t[:], d["W8S"][Jp])
            w8_pre[Jp] = w8_t
        ones = const.tile([1, 128], fp16)
        nc.vector.memset(ones[:], 1.0)
        zero = const.tile([128, 1], fp16)
        nc.vector.memset(zero[:], 0.0)
        # one-hot of x, built on device
        ohall = const.tile([MOD, 2 * BL * T], fp16)
        ohsb = [ohall[:, 0:BL * T], ohall[:, BL * T:2 * BL * T]]

        psum_h1 = ps_h1.tile([128, 512], fp32)
        psum_8 = ps_8.tile([128, 512], fp32)
        bias_done = [False]

        asb = {}

        def a_scan_open(j):
            a_sb = a_pool.tile([128, FREE], fp16, tag="a")
            sep = a_sb[:].rearrange("p (b t) -> p b t", t=CL)[:, :, T]
            nc.gpsimd.memset(sep, NEG)
            asb[j] = a_sb

        def a_scan_part(j, q):
            dd, m = j // 4, j % 4
            a_sb = asb[j]
            lhsT = wfe[:, dd * HID + m * 128: dd * HID + m * 128 + 128]
            if m == 0:
                px = ps_a.tile([128, 512], fp32, tag="pa")
                nc.tensor.matmul(
                    px[:MOD, :], ones[:, 0:MOD],
                    xr[:, dd * BL * T + q * 512: dd * BL * T + (q + 1) * 512],
                    start=True, stop=True,
                )
                nc.vector.tensor_tensor(
                    ohsb[dd][:, q * 512:(q + 1) * 512], px[:MOD, :],
                    arn[:MOD, :].broadcast_to([MOD, 512]),
                    op=mybir.AluOpType.is_equal,
                )
            pa = ps_a.tile([128, 512], fp32, tag="pa")
            nc.tensor.matmul(
                pa[:], lhsT, ohsb[dd][:, q * 512:(q + 1) * 512],
                start=True, stop=True,
            )
            av = a_sb[:].rearrange("p (b t) -> p b t", t=CL)[:, 16 * q:16 * q + 16, 0:T]
            nc.scalar.copy(av, pa[:].rearrange("p (b t) -> p b t", t=T))

        def a_scan_close(j):
            h_t = h_pool.tile([128, FREE], fp16, tag="h")
            nc.vector.tensor_tensor_scan(
                h_t[:], asb.pop(j)[:], zero[:].broadcast_to([128, FREE]),
                initial=0.0, op0=ALU.add, op1=ALU.max,
            )
            hs[j] = h_t

        def x_cast(j):
            # fp8 copies of the early-t states for the DoubleRow path:
            # X1[p, t*128 + b] = fp8(32 * h[p, b*33 + t]), t < TAU; the
            # residual X2 = 32*x - X1 captures exactly the device cast
            # error of X1 (same product scale -> same PSUM bank).
            h_t = hs[j]
            hv3 = h_t[:].rearrange("p (b t) -> p t b", t=CL)[:, 0:TAU, :]
            x1 = x1_pool.tile([128, TAU * BL], fp8, tag="x1")
            nc.scalar.activation(
                x1[:].rearrange("p (t b) -> p t b", b=BL), hv3, AF.Copy, scale=SX,
            )
            # residual term X2 = 32*x - X1 captures exactly the device cast
            # error of X1. e5m2 (min normal 2^-14) keeps the small residuals
            # out of the PE's subnormal flush; 2 mantissa bits suffice for a
            # second-order term. Same product scale -> same PSUM bank.
            x2 = x1_pool.tile([128, TAU * BL], fp8e5, tag="x2")
            nc.vector.scalar_tensor_tensor(
                x2[:].rearrange("p (t b) -> p t b", b=BL), hv3, SX,
                x1[:].rearrange("p (t b) -> p t b", b=BL),
                op0=ALU.mult, op1=ALU.subtract,
            )
            xs[j] = (x1, x2)

        hs = {}
        xs = {}

        def a_scan(j):
            a_scan_open(j)
            for q in range(8):
                a_scan_part(j, q)
            a_scan_close(j)

        for j in range(4):
            a_scan(j)
        x_cast(0)
        x_cast(1)

        def fp8_block(j):
            # fp8 DoubleRow matmuls for this j (data prefetched early)
            w8_t = w8_pre.pop(j)
            x1, x2 = xs.pop(j)
            x1v = x1[:].rearrange("p (t b) -> p t b", b=BL)
            x2v = x2[:].rearrange("p (t b) -> p t b", b=BL)
            w8v = w8_t[:].rearrange("p (pr two c) -> p pr two c", two=2, c=512)
            for pr in range(NPR):
                nc.tensor.matmul(
                    psum_8[:], x1v[:, 2 * pr:2 * pr + 2, :], w8v[:, pr],
                    start=(j == 0 and pr == 0), stop=False,
                    perf_mode=PM.DoubleRow,
                )
                nc.tensor.matmul(
                    psum_8[:], x2v[:, 2 * pr:2 * pr + 2, :], w8v[:, pr],
                    start=False, stop=(j == 7 and pr == NPR - 1),
                    perf_mode=PM.DoubleRow,
                )
            if j == 7:
                # fold the fp8 partial back into the main accumulation,
                # just ahead of the final group's stop matmul
                s8 = const.tile([128, 512], fp16)
                nc.scalar.activation(s8[:], psum_8[:], AF.Copy, scale=SINV)
                nc.tensor.matmul(psum_h1[:], idsb, s8[:], start=False, stop=False)

        for j in range(8):
            hv = hs[j][:].rearrange("p (b t) -> p t b", t=CL)
            if j < 7:
                fp8_block(j)
            for G in range(G16_PER_J * j, G16_PER_J * (j + 1)):
                if G == W1_GRP - 1:
                    fp8_block(7)
                w_t = w1_pre.pop(G, None)
                if w_t is None:
                    w_t = w1_pool.tile([128, 2048], fp16, tag="w_t")
                last_grp = G == W1_GRP - 1
                if last_grp:
                    for c in range(4):
                        nc.sync.dma_start(w_t[:, c * 512:(c + 1) * 512],
                                          d["W1S"][G][:, c * 512:(c + 1) * 512])
                elif G > 1:
                    nc.sync.dma_start(w_t[:], d["W1S"][G])
                if not bias_done[0]:
                    nc.tensor.matmul(psum_h1[:], ones[:], b1sb,
                                     start=True, stop=False)
                    bias_done[0] = True
                for c in range(4):
                    t_idx = TAU + (G % G16_PER_J) * 4 + c
                    nc.tensor.matmul(
                        psum_h1[:], hv[:, t_idx, :], w_t[:, c * 512:(c + 1) * 512],
                        start=False, stop=(last_grp and c == 3),
                    )
                if G == G16_PER_J * j:
                    if j + 4 < 8:
                        a_scan(j + 4)
                    if j + 2 < 8:
                        x_cast(j + 2)
                        w8_t2 = w8_pool.tile([128, NPR * 1024], fp8, tag="w8_t")
                        nc.sync.dma_start(w8_t2[:], d["W8S"][j + 2])
                        w8_pre[j + 2] = w8_t2
        # tail-only weights ship after the W1 stream
        nc.sync.dma_start(w2o[:], d["W2O"][:])
        h1sb = const.tile([128, 512], fp16)
        nc.scalar.activation(h1sb[:], psum_h1[:], AF.Relu)

        # ---- transpose h1 to feature-major [512, 128] ----
        pt_a = ps_l.tile([128, 256], fp16, tag="pla")
        pt_b = ps_l.tile([128, 256], fp16, tag="plb")
        cur = hp_pool.tile([128, 512], fp16, tag="hp")
        for m in (0, 1):
            nc.tensor.transpose(
                pt_a[:, (m % 2) * 128:(m % 2) * 128 + 128],
                h1sb[:, m * 128:(m + 1) * 128], idsb[:])
        nc.scalar.copy(cur[:, 0:256], pt_a[:])
        for m in (2, 3):
            nc.tensor.transpose(
                pt_b[:, (m % 2) * 128:(m % 2) * 128 + 128],
                h1sb[:, m * 128:(m + 1) * 128], idsb[:])
        nc.vector.tensor_copy(cur[:, 256:512], pt_b[:])

        # ---- 4 x (h = relu(W2 @ h' + b2)), feature-major, col block = m ----
        for _L in range(4):
            pl_a = ps_l.tile([128, 256], fp32, tag="pla")
            pl_b = ps_l.tile([128, 256], fp32, tag="plb")
            for m in range(4):
                pl = pl_a if m < 2 else pl_b
                col = (m % 2) * 128
                nc.tensor.matmul(
                    pl[:, col:col + 128],
                    b2r[:, m * 128:(m + 1) * 128], ones[:],
                    start=True, stop=False,
                )
                for k in range(4):
                    nc.tensor.matmul(
                        pl[:, col:col + 128],
                        w2sb[:, k * 512 + m * 128: k * 512 + m * 128 + 128],
                        cur[:, k * 128:(k + 1) * 128],
                        start=False, stop=(k == 3),
                    )
            hq = hp_pool.tile([128, 512], fp16, tag="hp")
            nc.scalar.activation(hq[:, 0:256], pl_a[:], AF.Relu)
            nc.vector.tensor_scalar_max(hq[:, 256:512], pl_b[:], 0.0)
            cur = hq

        # ---- output head: out' = Wo @ h' + bo  -> [97, 128] ----
        po = ps_o.tile([MOD, 128], fp32, tag="po")
        nc.tensor.matmul(po[:], bor, ones[:], start=True, stop=False)
        for k in range(4):
            nc.tensor.matmul(
                po[:], wosb[:, k * MOD:(k + 1) * MOD], cur[:, k * 128:(k + 1) * 128],
                start=False, stop=(k == 3),
            )
        osb = const.tile([MOD, BL], fp32)
        nc.scalar.copy(osb[:], po[:])
        nc.sync.dma_start(d["OUT"], osb[:])


def _host_prep(inputs):
    x = np.asarray(inputs["x"]).astype(np.int64)          # [B, T]
    emb = np.asarray(inputs["emb"], np.float32)           # [97, 512]
    Wf = np.asarray(inputs["Wf"], np.float32)
    bf = np.asarray(inputs["bf"], np.float32)
    Wb = np.asarray(inputs["Wb"], np.float32)
    bb = np.asarray(inputs["bb"], np.float32)
    W1 = np.asarray(inputs["W1"], np.float32)             # [512, 32768]
    b1 = np.asarray(inputs["b1"], np.float32)
    W2 = np.asarray(inputs["W2"], np.float32)
    b2 = np.asarray(inputs["b2"], np.float32)
    Wo = np.asarray(inputs["Wo"], np.float32)             # [97, 512]
    bo = np.asarray(inputs["bo"], np.float32)

    WFE = np.ascontiguousarray(np.stack([
        (Wf @ emb.T + bf[:, None]).T,                     # [97, 512]
        (Wb @ emb.T + bb[:, None]).T,
    ]).transpose(1, 0, 2).reshape(MOD, 2 * HID)).astype(F16)

    xc = x.reshape(NCORES, BL, T)
    XR = np.concatenate([
        xc.reshape(NCORES, BL * T), xc[:, :, ::-1].reshape(NCORES, BL * T)
    ], axis=1).astype(F16)                                # [NC, 8192]
    IDA = np.concatenate([
        np.eye(128, dtype=np.float32),
        np.arange(128, dtype=np.float32).reshape(128, 1),
    ], axis=1).astype(F16)

    # W1.T row layout is [t, d, m, p]-major (xcat col = t*1024 + d*512 + m*128)
    W1t = W1.T.reshape(T, 2, 4, 128, 512)                 # [t, d, m, p, col]
    # fp16 part: t in [TAU, 32): group G = (d, m, tg) holds t-chunks
    # t = TAU + 4*tg .. TAU + 4*tg + 3 side by side
    W1S = np.ascontiguousarray(
        W1t[TAU:]                                         # [NT16, d, m, p, col]
        .reshape(G16_PER_J, 4, 2, 4, 128, 512)            # [tg, tc, d, m, p, col]
        .transpose(2, 3, 0, 4, 1, 5)                      # [d, m, tg, p, tc, col]
        .reshape(W1_GRP, 128, 2048)
    ).astype(F16)
    # fp8 part: t in [0, TAU) as DoubleRow pairs, scaled by SW
    W8S = np.ascontiguousarray(
        (W1t[:TAU] * SW)                                  # [TAU, d, m, p, col]
        .reshape(NPR, 2, 2, 4, 128, 512)                  # [pr, two, d, m, p, col]
        .transpose(2, 3, 4, 0, 1, 5)                      # [d, m, p, pr, two, col]
        .reshape(8, 128, NPR * 1024)
    ).astype(E4)
    W2S = np.ascontiguousarray(W2.T.reshape(4, 128, 512).transpose(1, 0, 2).reshape(128, 2048)).astype(F16)
    WOS = np.ascontiguousarray(Wo.T.reshape(4, 128, MOD).transpose(1, 0, 2).reshape(128, 4 * MOD)).astype(F16)
    W2O = np.concatenate([W2S, WOS], axis=1)
    BIAH = np.concatenate([b1, b2, bo]).astype(F16)       # [1121]

    WFI = np.concatenate([
        np.concatenate([WFE, np.zeros((128 - MOD, 2 * HID), F16)], axis=0), IDA,
    ], axis=1)
    shared = {"WFI": WFI, "W1S": W1S, "W8S": W8S, "W2O": W2O}
    in_maps = [
        dict(shared, BIA=np.concatenate([BIAH, XR[c]]).reshape(1, -1))
        for c in range(NCORES)
    ]
    return in_maps


def _get_nc():
    if "nc" not in _CACHE:
        _CACHE["nc"] = _build()
    return _CACHE["nc"]


def kernel(**inputs):
    from concourse.bass_utils import run_bass_kernel_spmd

    nc = _get_nc()
    in_maps = _host_prep(inputs)
    res = run_bass_kernel_spmd(nc, in_maps, list(range(NCORES)))
    outs = [np.asarray(res.results[c]["OUT"], np.float32) for c in range(NCORES)]
    return np.ascontiguousarray(np.concatenate([o.T for o in outs], axis=0))  # [1024, 97]
